# revision 2
# baseline (speedup 1.0000x reference)
"""MobiuAttention Trainium2 kernel (8 NeuronCores, SPMD).

Sharding: core i handles (batch b = i//2, head-group g = i%2) -> 8 local heads.
Per core: fp32r projections, complexity sensor, chunked linear-attention
recurrence (chunk C=128, log-space cumulative decay, head-PAIR packed on the
128 partitions, fp32 matmuls), o_proj partial with the local head-slice of
o_w. Host sums the two partial y's per batch.
"""
import sys
sys.path.insert(0, '/opt/trn_rl_repo')

import numpy as np
import bass_rust
import concourse.bass as bass
import concourse.mybir as mybir
import concourse.tile as tile
from concourse.bass_utils import run_bass_kernel_spmd
from concourse.masks import make_identity, make_upper_triangular

F32 = mybir.dt.float32
F32R = mybir.dt.float32r
U8 = mybir.dt.uint8
BF16 = mybir.dt.bfloat16
AL = mybir.AluOpType
AF = mybir.ActivationFunctionType

B, T, D, H, E = 4, 2048, 1024, 16, 64
DH = D // 4          # 256 sensor hidden
HL = 8               # heads per core
NP = HL // 2         # 4 head pairs
DL = HL * E          # 512 local head dim
SC = 8               # superchunks
TC = T // SC         # 256 tokens per superchunk
C = 128              # recurrence chunk
NT = TC // C         # 2 chunks per superchunk
NDT = D // 128       # 8 contraction tiles
LOGCLIP = float(np.log(0.9995))

SEQ_ENGINES = {mybir.EngineType.PE, mybir.EngineType.DVE, mybir.EngineType.Activation,
               mybir.EngineType.Pool, mybir.EngineType.SP}


def _split_multiwait(nc, max_waits=1):
    """Walrus here encodes at most one sync-wait per instruction; hoist extra
    waits onto single-wait NOPs just before, on the same in-order sequencer."""
    for f in nc.m.functions:
        for bb in f.blocks:
            changed = False
            newlist = []
            for inst in bb.instructions:
                si = inst.sync_info
                if (si is not None and len(si.on_wait) > max_waits
                        and inst.engine in SEQ_ENGINES):
                    waits = list(si.on_wait)
                    for w in waits[:-1]:
                        nop = mybir.InstNoOp(name=nc.get_next_instruction_name(),
                                             ins=[], outs=[])
                        nop.engine = inst.engine
                        nop.sync_info = bass_rust.SyncInfo(on_wait=[w], on_update=[])
                        newlist.append(nop)
                        nc.register_instruction(nop)
                    inst.sync_info = bass_rust.SyncInfo(
                        on_wait=[waits[-1]], on_update=list(si.on_update))
                    changed = True
                newlist.append(inst)
            if changed:
                bb.instructions = newlist


def _build():
    nc = bass.Bass(trn_type="TRN2", num_devices=8)
    xT_d = nc.dram_tensor("xT", [128, NDT * T], F32R, kind="ExternalInput")
    wq_d = nc.dram_tensor("wq", [128, NDT * DL], F32R, kind="ExternalInput")
    wk_d = nc.dram_tensor("wk", [128, NDT * DL], F32R, kind="ExternalInput")
    wv_d = nc.dram_tensor("wv", [128, NDT * DL], F32R, kind="ExternalInput")
    wo_d = nc.dram_tensor("wo", [128, 4 * D], F32R, kind="ExternalInput")
    cs1_d = nc.dram_tensor("cs1", [128, NDT * DH], F32R, kind="ExternalInput")
    cs2_d = nc.dram_tensor("cs2", [128, 2 * HL], F32R, kind="ExternalInput")
    b1_d = nc.dram_tensor("b1", [128, 2], F32, kind="ExternalInput")
    b2_d = nc.dram_tensor("b2", [128, HL], F32, kind="ExternalInput")
    lb_d = nc.dram_tensor("lb", [128, DL], F32, kind="ExternalInput")
    y_d = nc.dram_tensor("y", [T, D], F32, kind="ExternalOutput")

    with tile.TileContext(nc) as tc:
        with tc.tile_pool(name="wpool", bufs=1) as wpool, \
             tc.tile_pool(name="cpool", bufs=1) as cpool, \
             tc.tile_pool(name="state", bufs=1) as state, \
             tc.tile_pool(name="xpool", bufs=2) as xpool, \
             tc.tile_pool(name="qkv", bufs=2) as qkv, \
             tc.tile_pool(name="hpool", bufs=2) as hpool, \
             tc.tile_pool(name="upool", bufs=2) as upool, \
             tc.tile_pool(name="otpool", bufs=2) as otpool, \
             tc.tile_pool(name="ypool", bufs=2) as ypool, \
             tc.tile_pool(name="rec", bufs=3) as rec, \
             tc.tile_pool(name="small", bufs=4) as small, \
             tc.tile_pool(name="psA", bufs=1, space="PSUM") as psA, \
             tc.tile_pool(name="psB", bufs=2, space="PSUM") as psB:

            # ---- constants ----
            ident = cpool.tile([128, 128], F32)
            make_identity(nc, ident[:])
            tri = cpool.tile([128, 128], F32)
            make_upper_triangular(nc, tri[:], val=1.0, diag=True)
            tri_u8 = cpool.tile([128, 128], U8)
            nc.vector.tensor_copy(tri_u8[:], tri[:])
            z128 = cpool.tile([128, 128], F32)
            nc.vector.memset(z128[:], 0.0)
            z128b = cpool.tile([128, 128], BF16)
            nc.vector.memset(z128b[:], 0.0)

            # ---- weights ----
            wq = wpool.tile([128, NDT * DL], F32R)
            nc.sync.dma_start(wq[:], wq_d[:])
            wk = wpool.tile([128, NDT * DL], F32R)
            nc.sync.dma_start(wk[:], wk_d[:])
            wv = wpool.tile([128, NDT * DL], F32R)
            nc.sync.dma_start(wv[:], wv_d[:])
            wo = wpool.tile([128, 4 * D], F32R)
            nc.sync.dma_start(wo[:], wo_d[:])
            cs1 = wpool.tile([128, NDT * DH], F32R)
            nc.sync.dma_start(cs1[:], cs1_d[:])
            cs2 = wpool.tile([128, 2 * HL], F32R)
            nc.sync.dma_start(cs2[:], cs2_d[:])
            b1 = wpool.tile([128, 2], F32)
            nc.sync.dma_start(b1[:], b1_d[:])
            b2 = wpool.tile([128, HL], F32)
            nc.sync.dma_start(b2[:], b2_d[:])
            lb = wpool.tile([128, DL], F32)
            nc.sync.dma_start(lb[:], lb_d[:])

            # ---- per-pair recurrent state [ (h0 e | h1 e), f ] ----
            S = []
            for mo in range(NP):
                sh = state.tile([128, 64], F32, tag=f"S{mo}", name=f"S{mo}")
                nc.vector.memset(sh[:], 0.0)
                S.append(sh)

            for sc in range(SC):
                xt = xpool.tile([128, NDT * TC], F32R, tag="xt")
                for dt in range(NDT):
                    nc.sync.dma_start(
                        xt[:, dt * TC:(dt + 1) * TC],
                        xT_d[:, dt * T + sc * TC: dt * T + sc * TC + TC])

                # ---- Q,K -> per-pair [ (2x64 e), t(TC) ] ----
                q_et, k_et = [], []
                for name, w, dst in (("q", wq, q_et), ("k", wk, k_et)):
                    for mo in range(NP):
                        pp = psB.tile([128, TC], F32, tag="proj")
                        for dt in range(NDT):
                            nc.tensor.matmul(
                                pp[:],
                                w[:, dt * DL + mo * 128: dt * DL + (mo + 1) * 128],
                                xt[:, dt * TC:(dt + 1) * TC],
                                start=(dt == 0), stop=(dt == NDT - 1))
                        sb = qkv.tile([128, TC], F32, tag=f"{name}{mo}")
                        nc.vector.tensor_copy(sb[:], pp[:])
                        dst.append(sb)

                # ---- V -> [t(128 x NT), dout(DL)] ----
                v_te, v_bf = [], []
                for tt in range(NT):
                    pp = psB.tile([128, DL], F32, tag="proj")
                    for dt in range(NDT):
                        nc.tensor.matmul(
                            pp[:, 0:DL],
                            xt[:, dt * TC + tt * 128: dt * TC + (tt + 1) * 128],
                            wv[:, dt * DL:(dt + 1) * DL],
                            start=(dt == 0), stop=(dt == NDT - 1))
                    sb = qkv.tile([128, DL], F32, tag=f"v{tt}")
                    nc.vector.tensor_copy(sb[:], pp[:, 0:DL])
                    v_te.append(sb)
                    vb = qkv.tile([128, DL], BF16, tag=f"vb{tt}")
                    nc.vector.tensor_copy(vb[:], sb[:])
                    v_bf.append(vb)

                # ---- sensor ----
                hid = []
                for mo in range(2):
                    pp = psB.tile([128, TC], F32, tag="proj")
                    for dt in range(NDT):
                        nc.tensor.matmul(
                            pp[:],
                            cs1[:, dt * DH + mo * 128: dt * DH + (mo + 1) * 128],
                            xt[:, dt * TC:(dt + 1) * TC],
                            start=(dt == 0), stop=(dt == NDT - 1))
                    sb = hpool.tile([128, TC], F32R, tag=f"h{mo}")
                    nc.scalar.activation(sb[:], pp[:], AF.Tanh, bias=b1[:, mo:mo + 1])
                    hid.append(sb)

                u_tt = []
                for tt in range(NT):
                    pp = psA.tile([128, HL], F32, tag="lamT")
                    for k2 in range(2):
                        nc.tensor.matmul(
                            pp[:],
                            hid[k2][:, tt * 128:(tt + 1) * 128],
                            cs2[:, k2 * HL:(k2 + 1) * HL],
                            start=(k2 == 0), stop=(k2 == 1))
                    zb = small.tile([128, HL], F32, tag="zb")
                    nc.vector.tensor_add(zb[:], pp[:], b2[:])
                    lc = small.tile([128, HL], F32, tag="lc")
                    nc.scalar.activation(lc[:], zb[:], AF.Sigmoid)
                    uu = upool.tile([128, HL], F32, tag=f"u{tt}")
                    nc.scalar.activation(uu[:], lc[:], AF.Ln, bias=1.0, scale=0.2)
                    u_tt.append(uu)

                # ---- OT collector: [64 f, (h-local, t)] per pair ----
                OT = [otpool.tile([128, TC], F32R, tag=f"ot{mo}",
                                  name=f"OT{mo}_{sc}") for mo in range(NP)]

                # ---- recurrence: chunk x pair ----
                for tt in range(NT):
                    for mo in range(NP):
                        q_p = q_et[mo][:, tt * 128:(tt + 1) * 128]
                        k_p = k_et[mo][:, tt * 128:(tt + 1) * 128]
                        v_p = v_te[tt][:, mo * 128:(mo + 1) * 128]
                        vbf_p = v_bf[tt][:, mo * 128:(mo + 1) * 128]
                        # log-lambda [t, (2x64 e)] then transpose to pair-et
                        lam = rec.tile([128, 128], F32, tag="lam")
                        for j in range(2):
                            h = 2 * mo + j
                            nc.vector.tensor_scalar(
                                lam[:, j * 64:(j + 1) * 64],
                                lb[:, h * 64:(h + 1) * 64],
                                u_tt[tt][:, h:h + 1], LOGCLIP,
                                AL.add, AL.min)
                        lamT = psA.tile([128, 128], F32, tag="lamT")
                        nc.tensor.transpose(lamT[:], lam[:], ident[:])
                        L = rec.tile([128, 128], F32, tag="L")
                        nc.vector.tensor_tensor_scan(
                            L[:], lamT[:], z128[:], 0.0, AL.add, AL.add)

                        L127 = L[:, 127:128]
                        ccol = small.tile([128, 1], F32, tag="ccol")
                        nc.vector.tensor_scalar_mul(ccol[:], L127, 0.5)
                        cneg = small.tile([128, 1], F32, tag="cneg")
                        nc.vector.tensor_scalar_mul(cneg[:], L127, -0.5)
                        ec = small.tile([128, 1], F32, tag="ec")
                        nc.scalar.activation(ec[:], L127, AF.Exp, scale=0.5)
                        aend = small.tile([128, 1], F32, tag="aend")
                        nc.scalar.activation(aend[:], L127, AF.Exp)

                        eq = rec.tile([128, 128], F32, tag="eq")
                        nc.scalar.activation(eq[:], L[:], AF.Exp, bias=cneg[:])
                        ekc = rec.tile([128, 128], F32, tag="ekc")
                        nc.scalar.activation(ekc[:], L[:], AF.Exp, bias=ccol[:],
                                             scale=-1.0)
                        ek7 = rec.tile([128, 128], F32, tag="ek7")
                        nc.scalar.activation(ek7[:], L[:], AF.Exp, bias=L127,
                                             scale=-1.0)

                        qt = rec.tile([128, 128], BF16, tag="qt")
                        nc.vector.tensor_mul(qt[:], q_p, eq[:])
                        kt = rec.tile([128, 128], BF16, tag="kt")
                        nc.vector.tensor_mul(kt[:], k_p, ekc[:])
                        kh = rec.tile([128, 128], F32, tag="kh")
                        nc.vector.tensor_mul(kh[:], k_p, ek7[:])

                        # K-hat pair transpose -> [t, (2x64 e)]
                        khT = psA.tile([128, 128], F32, tag="khT")
                        nc.tensor.transpose(khT[:], kh[:], ident[:])
                        khTs = rec.tile([128, 128], F32, tag="khTs")
                        nc.vector.tensor_copy(khTs[:], khT[:])

                        # S_scaled (both heads)
                        ssc = rec.tile([128, 64], BF16, tag="ssc")
                        nc.vector.tensor_scalar_mul(ssc[:], S[mo][:], ec[:])

                        # state delta for the pair (block-diagonal valid)
                        sd = psA.tile([128, 128], F32, tag="sd")
                        nc.tensor.matmul(sd[:], khTs[:], v_p, start=True, stop=True)

                        op = psB.tile([128, 128], F32, tag="outT")
                        for j in range(2):
                            sl = slice(j * 64, (j + 1) * 64)
                            # intra-chunk attention for head h = 2*mo + j
                            at = psA.tile([128, 128], F32, tag="at")
                            nc.tensor.matmul(at[:], kt[sl, :], qt[sl, :],
                                             start=True, stop=True)
                            atm = rec.tile([128, 128], BF16, tag="atm")
                            nc.gpsimd.tensor_copy(atm[:], z128b[:])
                            nc.vector.copy_predicated(atm[:], tri_u8[:], at[:])

                            nc.tensor.matmul(op[sl, :],
                                             vbf_p[:, j * 64:(j + 1) * 64],
                                             atm[:], start=True, stop=False)
                            nc.tensor.matmul(op[sl, :], ssc[sl, :], qt[sl, :],
                                             start=False, stop=True)
                            # state update for head h
                            nc.vector.scalar_tensor_tensor(
                                S[mo][sl, :], S[mo][sl, :], aend[sl, :],
                                sd[sl, j * 64:(j + 1) * 64], AL.mult, AL.add)
                        nc.vector.tensor_copy(
                            OT[mo][:, tt * 128:(tt + 1) * 128], op[:])

                # ---- o_proj ----
                for tt in range(NT):
                    for no in range(2):
                        pp = psB.tile([128, 512], F32, tag="proj")
                        for mo in range(NP):
                            nc.tensor.matmul(
                                pp[:],
                                OT[mo][:, tt * 128:(tt + 1) * 128],
                                wo[:, mo * D + no * 512: mo * D + no * 512 + 512],
                                start=(mo == 0), stop=(mo == NP - 1))
                        ysb = ypool.tile([128, 512], F32, tag="y")
                        nc.vector.tensor_copy(ysb[:], pp[:])
                        nc.sync.dma_start(
                            y_d[sc * TC + tt * 128: sc * TC + (tt + 1) * 128,
                                no * 512:(no + 1) * 512],
                            ysb[:])
    _split_multiwait(nc)
    return nc


_NC = None

def _get_nc():
    global _NC
    if _NC is None:
        _NC = _build()
    return _NC


def _sigmoid(x):
    return 1.0 / (1.0 + np.exp(-x))


def kernel(x, q_w, k_w, v_w, o_w, cs_w1, cs_b1, cs_w2, cs_b2, decay_params):
    x = np.asarray(x, np.float32)
    nc = _get_nc()

    def wlay(wT_cols):  # [1024, M] -> [128, 8*M] (dt-major along free)
        return np.ascontiguousarray(
            wT_cols.reshape(NDT, 128, wT_cols.shape[1]).transpose(1, 0, 2)
            .reshape(128, -1))

    qwT = np.asarray(q_w, np.float32).T
    kwT = np.asarray(k_w, np.float32).T
    vwT = np.asarray(v_w, np.float32).T
    owT = np.asarray(o_w, np.float32).T
    cs1T = np.asarray(cs_w1, np.float32).T      # [1024, 256]
    cs2T = np.asarray(cs_w2, np.float32).T      # [256, 16]
    lbase = np.log(_sigmoid(np.asarray(decay_params, np.float32)))  # [H, E]
    b1c = np.ascontiguousarray(np.asarray(cs_b1, np.float32).reshape(2, 128).T)

    in_maps = []
    for i in range(8):
        b, g = i // 2, i % 2
        hs = g * HL
        xT = x[b].T                                            # [1024, 2048]
        xTl = np.ascontiguousarray(
            xT.reshape(NDT, 128, T).transpose(1, 0, 2).reshape(128, NDT * T))
        wo_loc = owT[hs * E:(hs + HL) * E, :]                  # [512, 1024]
        wol = np.ascontiguousarray(                            # [128, 4*1024]
            wo_loc.reshape(4, 128, D).transpose(1, 0, 2).reshape(128, 4 * D))
        cs2l = np.ascontiguousarray(
            cs2T[:, hs:hs + HL].reshape(2, 128, HL).transpose(1, 0, 2)
            .reshape(128, 2 * HL))
        in_maps.append({
            "xT": xTl,
            "wq": wlay(qwT[:, hs * E:(hs + HL) * E]),
            "wk": wlay(kwT[:, hs * E:(hs + HL) * E]),
            "wv": wlay(vwT[:, hs * E:(hs + HL) * E]),
            "wo": wol,
            "cs1": wlay(cs1T),
            "cs2": cs2l,
            "b1": b1c,
            "b2": np.ascontiguousarray(
                np.broadcast_to(np.asarray(cs_b2, np.float32)[hs:hs + HL],
                                (128, HL))),
            "lb": np.ascontiguousarray(
                np.broadcast_to(lbase[hs:hs + HL].reshape(1, DL), (128, DL))),
        })

    res = run_bass_kernel_spmd(nc, in_maps, core_ids=list(range(8)))
    global LAST_RESULT
    LAST_RESULT = res
    y = np.empty((B, T, D), np.float32)
    for b in range(B):
        y[b] = res.results[2 * b]["y"] + res.results[2 * b + 1]["y"]
    return y



# revision 5
# speedup vs baseline: 1.0189x; 1.0189x over previous
"""MobiuAttention Trainium2 kernel (8 NeuronCores, SPMD).

Sharding: core i handles (batch b = i//2, head-group g = i%2) -> 8 local heads.
Per core: fp32r projections, complexity sensor, chunked linear-attention
recurrence (chunk C=128, log-space cumulative decay, head-PAIR packed on the
128 partitions, fp32 matmuls), o_proj partial with the local head-slice of
o_w. Host sums the two partial y's per batch.
"""
import sys
sys.path.insert(0, '/opt/trn_rl_repo')

import numpy as np
import bass_rust
import concourse.bass as bass
import concourse.mybir as mybir
import concourse.tile as tile
from concourse.bass_utils import run_bass_kernel_spmd
from concourse.masks import make_identity, make_upper_triangular

F32 = mybir.dt.float32
F32R = mybir.dt.float32r
U8 = mybir.dt.uint8
BF16 = mybir.dt.bfloat16
AL = mybir.AluOpType
AF = mybir.ActivationFunctionType

B, T, D, H, E = 4, 2048, 1024, 16, 64
DH = D // 4          # 256 sensor hidden
HL = 8               # heads per core
NP = HL // 2         # 4 head pairs
DL = HL * E          # 512 local head dim
SC = 8               # superchunks
TC = T // SC         # 256 tokens per superchunk
C = 128              # recurrence chunk
NT = TC // C         # 2 chunks per superchunk
NDT = D // 128       # 8 contraction tiles
LOGCLIP = float(np.log(0.9995))

SEQ_ENGINES = {mybir.EngineType.PE, mybir.EngineType.DVE, mybir.EngineType.Activation,
               mybir.EngineType.Pool, mybir.EngineType.SP}


def _split_multiwait(nc, max_waits=1):
    """Walrus here encodes at most one sync-wait per instruction; hoist extra
    waits onto single-wait NOPs just before, on the same in-order sequencer."""
    for f in nc.m.functions:
        for bb in f.blocks:
            changed = False
            newlist = []
            for inst in bb.instructions:
                si = inst.sync_info
                if (si is not None and len(si.on_wait) > max_waits
                        and inst.engine in SEQ_ENGINES):
                    waits = list(si.on_wait)
                    for w in waits[:-1]:
                        nop = mybir.InstNoOp(name=nc.get_next_instruction_name(),
                                             ins=[], outs=[])
                        nop.engine = inst.engine
                        nop.sync_info = bass_rust.SyncInfo(on_wait=[w], on_update=[])
                        newlist.append(nop)
                        nc.register_instruction(nop)
                    inst.sync_info = bass_rust.SyncInfo(
                        on_wait=[waits[-1]], on_update=list(si.on_update))
                    changed = True
                newlist.append(inst)
            if changed:
                bb.instructions = newlist


def _build():
    nc = bass.Bass(trn_type="TRN2", num_devices=8)
    xT_d = nc.dram_tensor("xT", [128, NDT * T], F32R, kind="ExternalInput")
    wq_d = nc.dram_tensor("wq", [128, NDT * DL], F32R, kind="ExternalInput")
    wk_d = nc.dram_tensor("wk", [128, NDT * DL], F32R, kind="ExternalInput")
    wv_d = nc.dram_tensor("wv", [128, NDT * DL], F32R, kind="ExternalInput")
    wo_d = nc.dram_tensor("wo", [128, 4 * D], F32R, kind="ExternalInput")
    cs1_d = nc.dram_tensor("cs1", [128, NDT * DH], F32R, kind="ExternalInput")
    cs2_d = nc.dram_tensor("cs2", [128, 2 * HL], F32R, kind="ExternalInput")
    b1_d = nc.dram_tensor("b1", [128, 2], F32, kind="ExternalInput")
    b2_d = nc.dram_tensor("b2", [128, HL], F32, kind="ExternalInput")
    lb_d = nc.dram_tensor("lb", [128, DL], F32, kind="ExternalInput")
    y_d = nc.dram_tensor("y", [T, D], F32, kind="ExternalOutput")

    with tile.TileContext(nc) as tc:
        with tc.tile_pool(name="wpool", bufs=1) as wpool, \
             tc.tile_pool(name="cpool", bufs=1) as cpool, \
             tc.tile_pool(name="state", bufs=1) as state, \
             tc.tile_pool(name="xpool", bufs=2) as xpool, \
             tc.tile_pool(name="qkv", bufs=2) as qkv, \
             tc.tile_pool(name="hpool", bufs=2) as hpool, \
             tc.tile_pool(name="upool", bufs=2) as upool, \
             tc.tile_pool(name="otpool", bufs=2) as otpool, \
             tc.tile_pool(name="ypool", bufs=2) as ypool, \
             tc.tile_pool(name="rec", bufs=3) as rec, \
             tc.tile_pool(name="small", bufs=4) as small, \
             tc.tile_pool(name="psA", bufs=1, space="PSUM") as psA, \
             tc.tile_pool(name="psB", bufs=2, space="PSUM") as psB:

            # ---- constants ----
            ident = cpool.tile([128, 128], F32)
            make_identity(nc, ident[:])
            tri = cpool.tile([128, 128], F32)
            make_upper_triangular(nc, tri[:], val=1.0, diag=True)
            tri_u8 = cpool.tile([128, 128], U8)
            nc.vector.tensor_copy(tri_u8[:], tri[:])
            z128 = cpool.tile([128, 128], F32)
            nc.vector.memset(z128[:], 0.0)
            z128b = cpool.tile([128, 128], BF16)
            nc.vector.memset(z128b[:], 0.0)

            # ---- weights ----
            wq = wpool.tile([128, NDT * DL], F32R)
            nc.sync.dma_start(wq[:], wq_d[:])
            wk = wpool.tile([128, NDT * DL], F32R)
            nc.sync.dma_start(wk[:], wk_d[:])
            wv = wpool.tile([128, NDT * DL], F32R)
            nc.sync.dma_start(wv[:], wv_d[:])
            wo = wpool.tile([128, 4 * D], F32R)
            nc.sync.dma_start(wo[:], wo_d[:])
            cs1 = wpool.tile([128, NDT * DH], F32R)
            nc.sync.dma_start(cs1[:], cs1_d[:])
            cs2 = wpool.tile([128, 2 * HL], F32R)
            nc.sync.dma_start(cs2[:], cs2_d[:])
            b1 = wpool.tile([128, 2], F32)
            nc.sync.dma_start(b1[:], b1_d[:])
            b2 = wpool.tile([128, HL], F32)
            nc.sync.dma_start(b2[:], b2_d[:])
            lb = wpool.tile([128, DL], F32)
            nc.sync.dma_start(lb[:], lb_d[:])

            # ---- per-pair recurrent state [ (h0 e | h1 e), f ] ----
            S = []
            for mo in range(NP):
                sh = state.tile([128, 64], F32, tag=f"S{mo}", name=f"S{mo}")
                nc.vector.memset(sh[:], 0.0)
                S.append(sh)

            for sc in range(SC):
                xt = xpool.tile([128, NDT * TC], F32R, tag="xt")
                for dt in range(NDT):
                    nc.sync.dma_start(
                        xt[:, dt * TC:(dt + 1) * TC],
                        xT_d[:, dt * T + sc * TC: dt * T + sc * TC + TC])

                # ---- Q,K -> per-pair [ (2x64 e), t(TC) ] ----
                q_et, k_et = [], []
                for name, w, dst in (("q", wq, q_et), ("k", wk, k_et)):
                    for mo in range(NP):
                        pp = psB.tile([128, TC], F32, tag="proj")
                        for dt in range(NDT):
                            nc.tensor.matmul(
                                pp[:],
                                w[:, dt * DL + mo * 128: dt * DL + (mo + 1) * 128],
                                xt[:, dt * TC:(dt + 1) * TC],
                                start=(dt == 0), stop=(dt == NDT - 1))
                        sb = qkv.tile([128, TC], F32, tag=f"{name}{mo}")
                        nc.vector.tensor_copy(sb[:], pp[:])
                        dst.append(sb)

                # ---- V -> [t(128 x NT), dout(DL)] ----
                v_bf = []
                for tt in range(NT):
                    pp = psB.tile([128, DL], F32, tag="proj")
                    for dt in range(NDT):
                        nc.tensor.matmul(
                            pp[:, 0:DL],
                            xt[:, dt * TC + tt * 128: dt * TC + (tt + 1) * 128],
                            wv[:, dt * DL:(dt + 1) * DL],
                            start=(dt == 0), stop=(dt == NDT - 1))
                    vb = qkv.tile([128, DL], BF16, tag=f"vb{tt}")
                    nc.vector.tensor_copy(vb[:], pp[:, 0:DL])
                    v_bf.append(vb)

                # ---- sensor ----
                hid = []
                for mo in range(2):
                    pp = psB.tile([128, TC], F32, tag="proj")
                    for dt in range(NDT):
                        nc.tensor.matmul(
                            pp[:],
                            cs1[:, dt * DH + mo * 128: dt * DH + (mo + 1) * 128],
                            xt[:, dt * TC:(dt + 1) * TC],
                            start=(dt == 0), stop=(dt == NDT - 1))
                    sb = hpool.tile([128, TC], F32R, tag=f"h{mo}")
                    nc.scalar.activation(sb[:], pp[:], AF.Tanh, bias=b1[:, mo:mo + 1])
                    hid.append(sb)

                u_tt = []
                for tt in range(NT):
                    pp = psA.tile([128, HL], F32, tag="lamT")
                    for k2 in range(2):
                        nc.tensor.matmul(
                            pp[:],
                            hid[k2][:, tt * 128:(tt + 1) * 128],
                            cs2[:, k2 * HL:(k2 + 1) * HL],
                            start=(k2 == 0), stop=(k2 == 1))
                    zb = small.tile([128, HL], F32, tag="zb")
                    nc.vector.tensor_add(zb[:], pp[:], b2[:])
                    lc = small.tile([128, HL], F32, tag="lc")
                    nc.scalar.activation(lc[:], zb[:], AF.Sigmoid)
                    uu = upool.tile([128, HL], F32, tag=f"u{tt}")
                    nc.scalar.activation(uu[:], lc[:], AF.Ln, bias=1.0, scale=0.2)
                    u_tt.append(uu)

                # ---- OT collector: [64 f, (h-local, t)] per pair ----
                OT = [otpool.tile([128, TC], F32R, tag=f"ot{mo}",
                                  name=f"OT{mo}_{sc}") for mo in range(NP)]

                # ---- recurrence: chunk x pair ----
                for tt in range(NT):
                    for mo in range(NP):
                        q_p = q_et[mo][:, tt * 128:(tt + 1) * 128]
                        k_p = k_et[mo][:, tt * 128:(tt + 1) * 128]
                        vbf_p = v_bf[tt][:, mo * 128:(mo + 1) * 128]
                        # log-lambda [t, (2x64 e)] then transpose to pair-et
                        lam = rec.tile([128, 128], F32, tag="lam")
                        for j in range(2):
                            h = 2 * mo + j
                            nc.vector.tensor_scalar(
                                lam[:, j * 64:(j + 1) * 64],
                                lb[:, h * 64:(h + 1) * 64],
                                u_tt[tt][:, h:h + 1], LOGCLIP,
                                AL.add, AL.min)
                        lamT = psA.tile([128, 128], F32, tag="lamT")
                        nc.tensor.transpose(lamT[:], lam[:], ident[:])
                        L = rec.tile([128, 128], F32, tag="L")
                        nc.vector.tensor_tensor_scan(
                            L[:], lamT[:], z128[:], 0.0, AL.add, AL.add)

                        L127 = L[:, 127:128]
                        ccol = small.tile([128, 1], F32, tag="ccol")
                        nc.vector.tensor_scalar_mul(ccol[:], L127, 0.5)
                        cneg = small.tile([128, 1], F32, tag="cneg")
                        nc.vector.tensor_scalar_mul(cneg[:], L127, -0.5)
                        ec = small.tile([128, 1], F32, tag="ec")
                        nc.scalar.activation(ec[:], L127, AF.Exp, scale=0.5)
                        aend = small.tile([128, 1], F32, tag="aend")
                        nc.scalar.activation(aend[:], L127, AF.Exp)

                        eq = rec.tile([128, 128], F32, tag="eq")
                        nc.scalar.activation(eq[:], L[:], AF.Exp, bias=cneg[:])
                        ekc = rec.tile([128, 128], F32, tag="ekc")
                        nc.scalar.activation(ekc[:], L[:], AF.Exp, bias=ccol[:],
                                             scale=-1.0)

                        qt = rec.tile([128, 128], BF16, tag="qt")
                        nc.vector.tensor_mul(qt[:], q_p, eq[:])
                        kt = rec.tile([128, 128], BF16, tag="kt")
                        nc.vector.tensor_mul(kt[:], k_p, ekc[:])
                        kh = rec.tile([128, 128], F32, tag="kh")
                        nc.vector.tensor_scalar_mul(kh[:], kt[:], ec[:])

                        # K-hat pair transpose -> [t, (2x64 e)]
                        khT = psA.tile([128, 128], F32, tag="khT")
                        nc.tensor.transpose(khT[:], kh[:], ident[:])
                        khTs = rec.tile([128, 128], BF16, tag="khTs")
                        nc.scalar.activation(khTs[:], khT[:], AF.Copy)

                        # S_scaled (both heads)
                        ssc = rec.tile([128, 64], BF16, tag="ssc")
                        nc.vector.tensor_scalar_mul(ssc[:], S[mo][:], ec[:])

                        # state delta for the pair (block-diagonal valid)
                        sd = psA.tile([128, 128], F32, tag="sd")
                        nc.tensor.matmul(sd[:], khTs[:], vbf_p, start=True,
                                         stop=True)

                        op = psB.tile([128, 128], F32, tag="outT")
                        for j in range(2):
                            sl = slice(j * 64, (j + 1) * 64)
                            # intra-chunk attention for head h = 2*mo + j
                            at = psA.tile([128, 128], F32, tag="at")
                            nc.tensor.matmul(at[:], kt[sl, :], qt[sl, :],
                                             start=True, stop=True)
                            atm = rec.tile([128, 128], BF16, tag="atm")
                            nc.vector.select(atm[:], tri_u8[:], at[:], z128[:])

                            nc.tensor.matmul(op[sl, :],
                                             vbf_p[:, j * 64:(j + 1) * 64],
                                             atm[:], start=True, stop=False)
                            nc.tensor.matmul(op[sl, :], ssc[sl, :], qt[sl, :],
                                             start=False, stop=True)
                            # state update for head h
                            nc.vector.scalar_tensor_tensor(
                                S[mo][sl, :], S[mo][sl, :], aend[sl, :],
                                sd[sl, j * 64:(j + 1) * 64], AL.mult, AL.add)
                        nc.scalar.activation(
                            OT[mo][:, tt * 128:(tt + 1) * 128], op[:], AF.Copy)

                # ---- o_proj ----
                for tt in range(NT):
                    for no in range(2):
                        pp = psB.tile([128, 512], F32, tag="proj")
                        for mo in range(NP):
                            nc.tensor.matmul(
                                pp[:],
                                OT[mo][:, tt * 128:(tt + 1) * 128],
                                wo[:, mo * D + no * 512: mo * D + no * 512 + 512],
                                start=(mo == 0), stop=(mo == NP - 1))
                        ysb = ypool.tile([128, 512], F32, tag="y")
                        nc.vector.tensor_copy(ysb[:], pp[:])
                        nc.sync.dma_start(
                            y_d[sc * TC + tt * 128: sc * TC + (tt + 1) * 128,
                                no * 512:(no + 1) * 512],
                            ysb[:])
    _split_multiwait(nc)
    return nc


_NC = None

def _get_nc():
    global _NC
    if _NC is None:
        _NC = _build()
    return _NC


def _sigmoid(x):
    return 1.0 / (1.0 + np.exp(-x))


def kernel(x, q_w, k_w, v_w, o_w, cs_w1, cs_b1, cs_w2, cs_b2, decay_params):
    x = np.asarray(x, np.float32)
    nc = _get_nc()

    def wlay(wT_cols):  # [1024, M] -> [128, 8*M] (dt-major along free)
        return np.ascontiguousarray(
            wT_cols.reshape(NDT, 128, wT_cols.shape[1]).transpose(1, 0, 2)
            .reshape(128, -1))

    qwT = np.asarray(q_w, np.float32).T
    kwT = np.asarray(k_w, np.float32).T
    vwT = np.asarray(v_w, np.float32).T
    owT = np.asarray(o_w, np.float32).T
    cs1T = np.asarray(cs_w1, np.float32).T      # [1024, 256]
    cs2T = np.asarray(cs_w2, np.float32).T      # [256, 16]
    lbase = np.log(_sigmoid(np.asarray(decay_params, np.float32)))  # [H, E]
    b1c = np.ascontiguousarray(np.asarray(cs_b1, np.float32).reshape(2, 128).T)

    in_maps = []
    for i in range(8):
        b, g = i // 2, i % 2
        hs = g * HL
        xT = x[b].T                                            # [1024, 2048]
        xTl = np.ascontiguousarray(
            xT.reshape(NDT, 128, T).transpose(1, 0, 2).reshape(128, NDT * T))
        wo_loc = owT[hs * E:(hs + HL) * E, :]                  # [512, 1024]
        wol = np.ascontiguousarray(                            # [128, 4*1024]
            wo_loc.reshape(4, 128, D).transpose(1, 0, 2).reshape(128, 4 * D))
        cs2l = np.ascontiguousarray(
            cs2T[:, hs:hs + HL].reshape(2, 128, HL).transpose(1, 0, 2)
            .reshape(128, 2 * HL))
        in_maps.append({
            "xT": xTl,
            "wq": wlay(qwT[:, hs * E:(hs + HL) * E]),
            "wk": wlay(kwT[:, hs * E:(hs + HL) * E]),
            "wv": wlay(vwT[:, hs * E:(hs + HL) * E]),
            "wo": wol,
            "cs1": wlay(cs1T),
            "cs2": cs2l,
            "b1": b1c,
            "b2": np.ascontiguousarray(
                np.broadcast_to(np.asarray(cs_b2, np.float32)[hs:hs + HL],
                                (128, HL))),
            "lb": np.ascontiguousarray(
                np.broadcast_to(lbase[hs:hs + HL].reshape(1, DL), (128, DL))),
        })

    res = run_bass_kernel_spmd(nc, in_maps, core_ids=list(range(8)))
    global LAST_RESULT
    LAST_RESULT = res
    y = np.empty((B, T, D), np.float32)
    for b in range(B):
        y[b] = res.results[2 * b]["y"] + res.results[2 * b + 1]["y"]
    return y



# revision 6
# speedup vs baseline: 1.2005x; 1.1782x over previous
"""MobiuAttention Trainium2 kernel (8 NeuronCores, SPMD).

Sharding: core i handles (batch b = i//2, head-group g = i%2) -> 8 local heads.
Per core: fp32r projections, complexity sensor, chunked linear-attention
recurrence (chunk C=128, log-space cumulative decay, head-PAIR packed on the
128 partitions, fp32 matmuls), o_proj partial with the local head-slice of
o_w. Host sums the two partial y's per batch.
"""
import sys
sys.path.insert(0, '/opt/trn_rl_repo')

import numpy as np
import bass_rust
import concourse.bass as bass
import concourse.mybir as mybir
import concourse.tile as tile
from concourse.bass_utils import run_bass_kernel_spmd
from concourse.masks import make_identity, make_upper_triangular

F32 = mybir.dt.float32
F32R = mybir.dt.float32r
U8 = mybir.dt.uint8
BF16 = mybir.dt.bfloat16
AL = mybir.AluOpType
AF = mybir.ActivationFunctionType

B, T, D, H, E = 4, 2048, 1024, 16, 64
DH = D // 4          # 256 sensor hidden
HL = 8               # heads per core
NP = HL // 2         # 4 head pairs
DL = HL * E          # 512 local head dim
SC = 8               # superchunks
TC = T // SC         # 256 tokens per superchunk
C = 128              # recurrence chunk
NT = TC // C         # 2 chunks per superchunk
NDT = D // 128       # 8 contraction tiles
LOGCLIP = float(np.log(0.9995))

SEQ_ENGINES = {mybir.EngineType.PE, mybir.EngineType.DVE, mybir.EngineType.Activation,
               mybir.EngineType.Pool, mybir.EngineType.SP}


def _split_multiwait(nc, max_waits=1):
    """Walrus here encodes at most one sync-wait per instruction; hoist extra
    waits onto single-wait NOPs just before, on the same in-order sequencer."""
    for f in nc.m.functions:
        for bb in f.blocks:
            changed = False
            newlist = []
            for inst in bb.instructions:
                si = inst.sync_info
                if (si is not None and len(si.on_wait) > max_waits
                        and inst.engine in SEQ_ENGINES):
                    waits = list(si.on_wait)
                    for w in waits[:-1]:
                        nop = mybir.InstNoOp(name=nc.get_next_instruction_name(),
                                             ins=[], outs=[])
                        nop.engine = inst.engine
                        nop.sync_info = bass_rust.SyncInfo(on_wait=[w], on_update=[])
                        newlist.append(nop)
                        nc.register_instruction(nop)
                    inst.sync_info = bass_rust.SyncInfo(
                        on_wait=[waits[-1]], on_update=list(si.on_update))
                    changed = True
                newlist.append(inst)
            if changed:
                bb.instructions = newlist


def _build():
    nc = bass.Bass(trn_type="TRN2", num_devices=8)
    xT_d = nc.dram_tensor("xT", [128, NDT * T], F32R, kind="ExternalInput")
    wq_d = nc.dram_tensor("wq", [128, NDT * DL], F32R, kind="ExternalInput")
    wk_d = nc.dram_tensor("wk", [128, NDT * DL], F32R, kind="ExternalInput")
    wv_d = nc.dram_tensor("wv", [128, NDT * DL], F32R, kind="ExternalInput")
    wo_d = nc.dram_tensor("wo", [128, 4 * D], F32R, kind="ExternalInput")
    cs1_d = nc.dram_tensor("cs1", [128, NDT * DH], F32R, kind="ExternalInput")
    cs2_d = nc.dram_tensor("cs2", [128, 2 * HL], F32R, kind="ExternalInput")
    b1_d = nc.dram_tensor("b1", [128, 2], F32, kind="ExternalInput")
    b2_d = nc.dram_tensor("b2", [128, HL], F32, kind="ExternalInput")
    lb_d = nc.dram_tensor("lb", [128, DL], F32, kind="ExternalInput")
    y_d = nc.dram_tensor("y", [T, D], F32, kind="ExternalOutput")

    with tile.TileContext(nc) as tc:
        with tc.tile_pool(name="wpool", bufs=1) as wpool, \
             tc.tile_pool(name="cpool", bufs=1) as cpool, \
             tc.tile_pool(name="state", bufs=1) as state, \
             tc.tile_pool(name="xpool", bufs=2) as xpool, \
             tc.tile_pool(name="qkv", bufs=2) as qkv, \
             tc.tile_pool(name="hpool", bufs=2) as hpool, \
             tc.tile_pool(name="upool", bufs=2) as upool, \
             tc.tile_pool(name="otpool", bufs=2) as otpool, \
             tc.tile_pool(name="ypool", bufs=2) as ypool, \
             tc.tile_pool(name="rec", bufs=3) as rec, \
             tc.tile_pool(name="small", bufs=4) as small, \
             tc.tile_pool(name="psA", bufs=1, space="PSUM") as psA, \
             tc.tile_pool(name="psB", bufs=2, space="PSUM") as psB:

            # ---- constants ----
            ident = cpool.tile([128, 128], F32)
            make_identity(nc, ident[:])
            tri = cpool.tile([128, 128], F32)
            make_upper_triangular(nc, tri[:], val=1.0, diag=True)
            tri_u8 = cpool.tile([128, 128], U8)
            nc.vector.tensor_copy(tri_u8[:], tri[:])
            triH = cpool.tile([128, 128], F16)
            nc.vector.tensor_copy(triH[:], tri[:])
            z128 = cpool.tile([128, 128], F32)
            nc.vector.memset(z128[:], 0.0)

            # ---- weights ----
            wq = wpool.tile([128, NDT * DL], F32R)
            nc.sync.dma_start(wq[:], wq_d[:])
            wk = wpool.tile([128, NDT * DL], F32R)
            nc.sync.dma_start(wk[:], wk_d[:])
            wv = wpool.tile([128, NDT * DL], F32R)
            nc.sync.dma_start(wv[:], wv_d[:])
            wo = wpool.tile([128, 4 * D], F32R)
            nc.sync.dma_start(wo[:], wo_d[:])
            cs1 = wpool.tile([128, NDT * DH], F32R)
            nc.sync.dma_start(cs1[:], cs1_d[:])
            cs2 = wpool.tile([128, 2 * HL], F32R)
            nc.sync.dma_start(cs2[:], cs2_d[:])
            b1 = wpool.tile([128, 2], F32)
            nc.sync.dma_start(b1[:], b1_d[:])
            b2 = wpool.tile([128, HL], F32)
            nc.sync.dma_start(b2[:], b2_d[:])
            lb = wpool.tile([128, DL], F32)
            nc.sync.dma_start(lb[:], lb_d[:])

            # ---- per-pair recurrent state [ (h0 e | h1 e), f ] ----
            S = []
            for mo in range(NP):
                sh = state.tile([128, 128], F32, tag=f"S{mo}", name=f"S{mo}")
                nc.vector.memset(sh[:], 0.0)
                S.append(sh)

            for sc in range(SC):
                xt = xpool.tile([128, NDT * TC], F32R, tag="xt")
                for dt in range(NDT):
                    nc.sync.dma_start(
                        xt[:, dt * TC:(dt + 1) * TC],
                        xT_d[:, dt * T + sc * TC: dt * T + sc * TC + TC])

                # ---- Q,K -> per-pair [ (2x64 e), t(TC) ] ----
                q_et, k_et = [], []
                for name, w, dst in (("q", wq, q_et), ("k", wk, k_et)):
                    for mo in range(NP):
                        pp = psB.tile([128, TC], F32, tag="proj")
                        for dt in range(NDT):
                            nc.tensor.matmul(
                                pp[:],
                                w[:, dt * DL + mo * 128: dt * DL + (mo + 1) * 128],
                                xt[:, dt * TC:(dt + 1) * TC],
                                start=(dt == 0), stop=(dt == NDT - 1))
                        sb = qkv.tile([128, TC], F32, tag=f"{name}{mo}")
                        nc.vector.tensor_copy(sb[:], pp[:])
                        dst.append(sb)

                # ---- V -> [t(128 x NT), dout(DL)] ----
                v_bf = []
                for tt in range(NT):
                    pp = psB.tile([128, DL], F32, tag="proj")
                    for dt in range(NDT):
                        nc.tensor.matmul(
                            pp[:, 0:DL],
                            xt[:, dt * TC + tt * 128: dt * TC + (tt + 1) * 128],
                            wv[:, dt * DL:(dt + 1) * DL],
                            start=(dt == 0), stop=(dt == NDT - 1))
                    vb = qkv.tile([128, DL], BF16, tag=f"vb{tt}")
                    nc.vector.tensor_copy(vb[:], pp[:, 0:DL])
                    v_bf.append(vb)

                # ---- sensor ----
                hid = []
                for mo in range(2):
                    pp = psB.tile([128, TC], F32, tag="proj")
                    for dt in range(NDT):
                        nc.tensor.matmul(
                            pp[:],
                            cs1[:, dt * DH + mo * 128: dt * DH + (mo + 1) * 128],
                            xt[:, dt * TC:(dt + 1) * TC],
                            start=(dt == 0), stop=(dt == NDT - 1))
                    sb = hpool.tile([128, TC], F32R, tag=f"h{mo}")
                    nc.scalar.activation(sb[:], pp[:], AF.Tanh, bias=b1[:, mo:mo + 1])
                    hid.append(sb)

                u_tt = []
                for tt in range(NT):
                    pp = psA.tile([128, HL], F32, tag="lamT")
                    for k2 in range(2):
                        nc.tensor.matmul(
                            pp[:],
                            hid[k2][:, tt * 128:(tt + 1) * 128],
                            cs2[:, k2 * HL:(k2 + 1) * HL],
                            start=(k2 == 0), stop=(k2 == 1))
                    zb = small.tile([128, HL], F32, tag="zb")
                    nc.vector.tensor_add(zb[:], pp[:], b2[:])
                    lc = small.tile([128, HL], F32, tag="lc")
                    nc.scalar.activation(lc[:], zb[:], AF.Sigmoid)
                    uu = upool.tile([128, HL], F32, tag=f"u{tt}")
                    nc.scalar.activation(uu[:], lc[:], AF.Ln, bias=1.0, scale=0.2)
                    u_tt.append(uu)

                # ---- OT collector: [64 f, (h-local, t)] per pair ----
                OT = [otpool.tile([128, TC], F32R, tag=f"ot{mo}",
                                  name=f"OT{mo}_{sc}") for mo in range(NP)]

                # ---- recurrence: chunk x pair ----
                for tt in range(NT):
                    for mo in range(NP):
                        q_p = q_et[mo][:, tt * 128:(tt + 1) * 128]
                        k_p = k_et[mo][:, tt * 128:(tt + 1) * 128]
                        vbf_p = v_bf[tt][:, mo * 128:(mo + 1) * 128]
                        # log-lambda [t, (2x64 e)] fp16; L[e,t] = lam^T @ tri
                        lam = rec.tile([128, 128], F16, tag="lam")
                        for j in range(2):
                            h = 2 * mo + j
                            nc.vector.tensor_scalar(
                                lam[:, j * 64:(j + 1) * 64],
                                lb[:, h * 64:(h + 1) * 64],
                                u_tt[tt][:, h:h + 1], LOGCLIP,
                                AL.add, AL.min)
                        L = psA.tile([128, 128], F32, tag="lamT")
                        nc.tensor.matmul(L[:], lam[:], triH[:],
                                         start=True, stop=True)

                        L127 = L[:, 127:128]
                        ccol = small.tile([128, 1], F32, tag="ccol")
                        nc.vector.tensor_scalar_mul(ccol[:], L127, 0.5)
                        cneg = small.tile([128, 1], F32, tag="cneg")
                        nc.vector.tensor_scalar_mul(cneg[:], L127, -0.5)
                        ec = small.tile([128, 1], F32, tag="ec")
                        nc.scalar.activation(ec[:], L127, AF.Exp, scale=0.5)
                        eq = rec.tile([128, 128], F32, tag="eq")
                        nc.scalar.activation(eq[:], L[:], AF.Exp, bias=cneg[:])
                        ekc = rec.tile([128, 128], F32, tag="ekc")
                        nc.scalar.activation(ekc[:], L[:], AF.Exp, bias=ccol[:],
                                             scale=-1.0)

                        qt = rec.tile([128, 128], BF16, tag="qt")
                        nc.vector.tensor_mul(qt[:], q_p, eq[:])
                        kt = rec.tile([128, 128], BF16, tag="kt")
                        nc.vector.tensor_mul(kt[:], k_p, ekc[:])
                        kh = rec.tile([128, 128], F32, tag="kh")
                        nc.vector.tensor_scalar_mul(kh[:], kt[:], ec[:])

                        # K-hat pair transpose -> [t, (2x64 e)]
                        khT = psA.tile([128, 128], F32, tag="khT")
                        nc.tensor.transpose(khT[:], kh[:], ident[:])
                        khTs = rec.tile([128, 128], BF16, tag="khTs")
                        nc.scalar.activation(khTs[:], khT[:], AF.Copy)

                        # S_scaled (block-diagonal, both heads)
                        ssc = rec.tile([128, 128], BF16, tag="ssc")
                        nc.vector.tensor_scalar_mul(ssc[:], S[mo][:], ec[:])

                        # state delta for the pair (block-diagonal valid)
                        sd = psA.tile([128, 128], F32, tag="sd")
                        nc.tensor.matmul(sd[:], khTs[:], vbf_p, start=True,
                                         stop=True)

                        op = psB.tile([128, 128], F32, tag="outT")
                        for j in range(2):
                            sl = slice(j * 64, (j + 1) * 64)
                            # intra-chunk attention for head h = 2*mo + j
                            at = psA.tile([128, 128], F32, tag="at")
                            nc.tensor.matmul(at[:], kt[sl, :], qt[sl, :],
                                             start=True, stop=True)
                            atm = rec.tile([128, 128], BF16, tag="atm")
                            nc.vector.select(atm[:], tri_u8[:], at[:], z128[:])

                            nc.tensor.matmul(op[sl, :],
                                             vbf_p[:, j * 64:(j + 1) * 64],
                                             atm[:], start=True, stop=False,
                                             skip_group_check=True)
                        # inter-chunk term for BOTH heads via block-diag ssc
                        nc.tensor.matmul(op[:], ssc[:], qt[:], start=False,
                                         stop=True, skip_group_check=True)
                        nc.scalar.activation(
                            OT[mo][:, tt * 128:(tt + 1) * 128], op[:], AF.Copy)
                        for j in range(2):
                            sl = slice(j * 64, (j + 1) * 64)
                            cs = slice(j * 64, (j + 1) * 64)
                            # S = ec*(ssc) + sd = ec^2*S + sd (per head block)
                            nc.vector.scalar_tensor_tensor(
                                S[mo][sl, cs], ssc[sl, cs], ec[sl, :],
                                sd[sl, cs], AL.mult, AL.add)

                # ---- o_proj ----
                for tt in range(NT):
                    for no in range(2):
                        pp = psB.tile([128, 512], F32, tag="proj")
                        for mo in range(NP):
                            nc.tensor.matmul(
                                pp[:],
                                OT[mo][:, tt * 128:(tt + 1) * 128],
                                wo[:, mo * D + no * 512: mo * D + no * 512 + 512],
                                start=(mo == 0), stop=(mo == NP - 1))
                        ysb = ypool.tile([128, 512], F32, tag="y")
                        nc.vector.tensor_copy(ysb[:], pp[:])
                        nc.sync.dma_start(
                            y_d[sc * TC + tt * 128: sc * TC + (tt + 1) * 128,
                                no * 512:(no + 1) * 512],
                            ysb[:])
    _split_multiwait(nc)
    return nc


_NC = None

def _get_nc():
    global _NC
    if _NC is None:
        _NC = _build()
    return _NC


def _sigmoid(x):
    return 1.0 / (1.0 + np.exp(-x))


def kernel(x, q_w, k_w, v_w, o_w, cs_w1, cs_b1, cs_w2, cs_b2, decay_params):
    x = np.asarray(x, np.float32)
    nc = _get_nc()

    def wlay(wT_cols):  # [1024, M] -> [128, 8*M] (dt-major along free)
        return np.ascontiguousarray(
            wT_cols.reshape(NDT, 128, wT_cols.shape[1]).transpose(1, 0, 2)
            .reshape(128, -1))

    qwT = np.asarray(q_w, np.float32).T
    kwT = np.asarray(k_w, np.float32).T
    vwT = np.asarray(v_w, np.float32).T
    owT = np.asarray(o_w, np.float32).T
    cs1T = np.asarray(cs_w1, np.float32).T      # [1024, 256]
    cs2T = np.asarray(cs_w2, np.float32).T      # [256, 16]
    lbase = np.log(_sigmoid(np.asarray(decay_params, np.float32)))  # [H, E]
    b1c = np.ascontiguousarray(np.asarray(cs_b1, np.float32).reshape(2, 128).T)

    in_maps = []
    for i in range(8):
        b, g = i // 2, i % 2
        hs = g * HL
        xT = x[b].T                                            # [1024, 2048]
        xTl = np.ascontiguousarray(
            xT.reshape(NDT, 128, T).transpose(1, 0, 2).reshape(128, NDT * T))
        wo_loc = owT[hs * E:(hs + HL) * E, :]                  # [512, 1024]
        wol = np.ascontiguousarray(                            # [128, 4*1024]
            wo_loc.reshape(4, 128, D).transpose(1, 0, 2).reshape(128, 4 * D))
        cs2l = np.ascontiguousarray(
            cs2T[:, hs:hs + HL].reshape(2, 128, HL).transpose(1, 0, 2)
            .reshape(128, 2 * HL))
        in_maps.append({
            "xT": xTl,
            "wq": wlay(qwT[:, hs * E:(hs + HL) * E]),
            "wk": wlay(kwT[:, hs * E:(hs + HL) * E]),
            "wv": wlay(vwT[:, hs * E:(hs + HL) * E]),
            "wo": wol,
            "cs1": wlay(cs1T),
            "cs2": cs2l,
            "b1": b1c,
            "b2": np.ascontiguousarray(
                np.broadcast_to(np.asarray(cs_b2, np.float32)[hs:hs + HL],
                                (128, HL))),
            "lb": np.ascontiguousarray(
                np.broadcast_to(lbase[hs:hs + HL].reshape(1, DL), (128, DL))),
        })

    res = run_bass_kernel_spmd(nc, in_maps, core_ids=list(range(8)))
    global LAST_RESULT
    LAST_RESULT = res
    y = np.empty((B, T, D), np.float32)
    for b in range(B):
        y[b] = res.results[2 * b]["y"] + res.results[2 * b + 1]["y"]
    return y



# revision 7
# speedup vs baseline: 1.6141x; 1.3445x over previous
"""MobiuAttention Trainium2 kernel (8 NeuronCores, SPMD).

Sharding: core i handles (batch b = i//2, head-group g = i%2) -> 8 local heads.
Per core: fp32r projections, complexity sensor, chunked linear-attention
recurrence (chunk C=128, log-space cumulative decay, head-PAIR packed on the
128 partitions, fp32 matmuls), o_proj partial with the local head-slice of
o_w. Host sums the two partial y's per batch.
"""
import sys
sys.path.insert(0, '/opt/trn_rl_repo')

import numpy as np
import bass_rust
import concourse.bass as bass
import concourse.mybir as mybir
import concourse.tile as tile
from concourse.bass_utils import run_bass_kernel_spmd
from concourse.masks import make_identity, make_upper_triangular

F32 = mybir.dt.float32
F32R = mybir.dt.float32r
U8 = mybir.dt.uint8
BF16 = mybir.dt.bfloat16
AL = mybir.AluOpType
AF = mybir.ActivationFunctionType

B, T, D, H, E = 4, 2048, 1024, 16, 64
DH = D // 4          # 256 sensor hidden
HL = 8               # heads per core
NP = HL // 2         # 4 head pairs
DL = HL * E          # 512 local head dim
SC = 8               # superchunks
TC = T // SC         # 256 tokens per superchunk
C = 128              # recurrence chunk
NT = TC // C         # 2 chunks per superchunk
NDT = D // 128       # 8 contraction tiles
LOGCLIP = float(np.log(0.9995))

SEQ_ENGINES = {mybir.EngineType.PE, mybir.EngineType.DVE, mybir.EngineType.Activation,
               mybir.EngineType.Pool, mybir.EngineType.SP}


def _split_multiwait(nc, max_waits=1):
    """Walrus here encodes at most one sync-wait per instruction; hoist extra
    waits onto single-wait NOPs just before, on the same in-order sequencer."""
    for f in nc.m.functions:
        for bb in f.blocks:
            changed = False
            newlist = []
            for inst in bb.instructions:
                si = inst.sync_info
                if (si is not None and len(si.on_wait) > max_waits
                        and inst.engine in SEQ_ENGINES):
                    waits = list(si.on_wait)
                    for w in waits[:-1]:
                        nop = mybir.InstNoOp(name=nc.get_next_instruction_name(),
                                             ins=[], outs=[])
                        nop.engine = inst.engine
                        nop.sync_info = bass_rust.SyncInfo(on_wait=[w], on_update=[])
                        newlist.append(nop)
                        nc.register_instruction(nop)
                    inst.sync_info = bass_rust.SyncInfo(
                        on_wait=[waits[-1]], on_update=list(si.on_update))
                    changed = True
                newlist.append(inst)
            if changed:
                bb.instructions = newlist


def _build():
    nc = bass.Bass(trn_type="TRN2", num_devices=8)
    xT_d = nc.dram_tensor("xT", [128, NDT * T], F32R, kind="ExternalInput")
    wq_d = nc.dram_tensor("wq", [128, NDT * DL], F32R, kind="ExternalInput")
    wk_d = nc.dram_tensor("wk", [128, NDT * DL], F32R, kind="ExternalInput")
    wv_d = nc.dram_tensor("wv", [128, NDT * DL], F32R, kind="ExternalInput")
    wo_d = nc.dram_tensor("wo", [128, 4 * D], F32R, kind="ExternalInput")
    cs1_d = nc.dram_tensor("cs1", [128, NDT * DH], F32R, kind="ExternalInput")
    cs2_d = nc.dram_tensor("cs2", [128, 2 * HL], F32R, kind="ExternalInput")
    b1_d = nc.dram_tensor("b1", [128, 2], F32, kind="ExternalInput")
    b2_d = nc.dram_tensor("b2", [128, HL], F32, kind="ExternalInput")
    lb_d = nc.dram_tensor("lb", [128, DL], F32, kind="ExternalInput")
    y_d = nc.dram_tensor("y", [T, D], F32, kind="ExternalOutput")

    with tile.TileContext(nc) as tc:
        with tc.tile_pool(name="wpool", bufs=1) as wpool, \
             tc.tile_pool(name="cpool", bufs=1) as cpool, \
             tc.tile_pool(name="state", bufs=1) as state, \
             tc.tile_pool(name="xpool", bufs=2) as xpool, \
             tc.tile_pool(name="qkv", bufs=2) as qkv, \
             tc.tile_pool(name="hpool", bufs=2) as hpool, \
             tc.tile_pool(name="upool", bufs=2) as upool, \
             tc.tile_pool(name="otpool", bufs=2) as otpool, \
             tc.tile_pool(name="ypool", bufs=2) as ypool, \
             tc.tile_pool(name="rec", bufs=3) as rec, \
             tc.tile_pool(name="small", bufs=4) as small, \
             tc.tile_pool(name="psA", bufs=1, space="PSUM") as psA, \
             tc.tile_pool(name="psB", bufs=2, space="PSUM") as psB:

            # ---- constants ----
            ident = cpool.tile([128, 128], F32)
            make_identity(nc, ident[:])
            tri = cpool.tile([128, 128], F32)
            make_upper_triangular(nc, tri[:], val=1.0, diag=True)
            tri_u8 = cpool.tile([128, 128], U8)
            nc.vector.tensor_copy(tri_u8[:], tri[:])
            triH = cpool.tile([128, 128], F16)
            nc.vector.tensor_copy(triH[:], tri[:])
            z128 = cpool.tile([128, 128], F32)
            nc.vector.memset(z128[:], 0.0)

            # ---- weights ----
            wq = wpool.tile([128, NDT * DL], F32R)
            nc.sync.dma_start(wq[:], wq_d[:])
            wk = wpool.tile([128, NDT * DL], F32R)
            nc.sync.dma_start(wk[:], wk_d[:])
            wv = wpool.tile([128, NDT * DL], F32R)
            nc.sync.dma_start(wv[:], wv_d[:])
            wo = wpool.tile([128, 4 * D], F32R)
            nc.sync.dma_start(wo[:], wo_d[:])
            cs1 = wpool.tile([128, NDT * DH], F32R)
            nc.sync.dma_start(cs1[:], cs1_d[:])
            cs2 = wpool.tile([128, 2 * HL], F32R)
            nc.sync.dma_start(cs2[:], cs2_d[:])
            b1 = wpool.tile([128, 2], F32)
            nc.sync.dma_start(b1[:], b1_d[:])
            b2 = wpool.tile([128, HL], F32)
            nc.sync.dma_start(b2[:], b2_d[:])
            lb = wpool.tile([128, DL], F32)
            nc.sync.dma_start(lb[:], lb_d[:])

            # ---- per-pair recurrent state [ (h0 e | h1 e), f ] ----
            S = []
            for mo in range(NP):
                sh = state.tile([128, 128], F32, tag=f"S{mo}", name=f"S{mo}")
                nc.vector.memset(sh[:], 0.0)
                S.append(sh)

            # ---- per-superchunk tile stores ----
            xt_t, q_et_t, k_et_t, v_bf_t = {}, {}, {}, {}
            hid_t, u_tt_t, OT_t = {}, {}, {}

            def load_x(sc):
                xt = xpool.tile([128, NDT * TC], F16, tag="xt", name=f"xt{sc}")
                for dt in range(NDT):
                    nc.sync.dma_start(
                        xt[:, dt * TC:(dt + 1) * TC],
                        xT_d[:, dt * T + sc * TC: dt * T + sc * TC + TC])
                xt_t[sc] = xt

            def proj_steps(sc):
                steps = []
                q_et_t[sc] = [None] * NP
                k_et_t[sc] = [None] * NP
                v_bf_t[sc] = [None] * NT
                hid_t[sc] = [None] * 2
                u_tt_t[sc] = [None] * NT

                def qk_step(name, w, store, mo):
                    def go():
                        xt = xt_t[sc]
                        pp = psB.tile([128, TC], F32, tag="proj",
                                      name=f"pp{name}{mo}_{sc}")
                        for dt in range(NDT):
                            nc.tensor.matmul(
                                pp[:],
                                w[:, dt * DL + mo * 128: dt * DL + (mo + 1) * 128],
                                xt[:, dt * TC:(dt + 1) * TC],
                                start=(dt == 0), stop=(dt == NDT - 1))
                        sb = qkv.tile([128, TC], F16, tag=f"{name}{mo}",
                                      name=f"{name}{mo}_{sc}")
                        nc.vector.tensor_copy(sb[:], pp[:])
                        store[sc][mo] = sb
                    return go

                for mo in range(NP):
                    steps.append(qk_step("q", wq, q_et_t, mo))
                for mo in range(NP):
                    steps.append(qk_step("k", wk, k_et_t, mo))

                def v_step(tt):
                    def go():
                        xt = xt_t[sc]
                        pp = psB.tile([128, DL], F32, tag="proj",
                                      name=f"ppv{tt}_{sc}")
                        for dt in range(NDT):
                            nc.tensor.matmul(
                                pp[:, 0:DL],
                                xt[:, dt * TC + tt * 128: dt * TC + (tt + 1) * 128],
                                wv[:, dt * DL:(dt + 1) * DL],
                                start=(dt == 0), stop=(dt == NDT - 1))
                        vb = qkv.tile([128, DL], BF16, tag=f"vb{tt}",
                                      name=f"vb{tt}_{sc}")
                        nc.vector.tensor_copy(vb[:], pp[:, 0:DL])
                        v_bf_t[sc][tt] = vb
                    return go

                for tt in range(NT):
                    steps.append(v_step(tt))

                def hid_step(mo):
                    def go():
                        xt = xt_t[sc]
                        pp = psB.tile([128, TC], F32, tag="proj",
                                      name=f"pph{mo}_{sc}")
                        for dt in range(NDT):
                            nc.tensor.matmul(
                                pp[:],
                                cs1[:, dt * DH + mo * 128: dt * DH + (mo + 1) * 128],
                                xt[:, dt * TC:(dt + 1) * TC],
                                start=(dt == 0), stop=(dt == NDT - 1))
                        sb = hpool.tile([128, TC], F16, tag=f"h{mo}",
                                        name=f"h{mo}_{sc}")
                        nc.scalar.activation(sb[:], pp[:], AF.Tanh,
                                             bias=b1[:, mo:mo + 1])
                        hid_t[sc][mo] = sb
                    return go

                steps.append(hid_step(0))
                steps.append(hid_step(1))

                def u_step():
                    # u-mms for both chunks, then Sigmoids adjacent, then Lns
                    # adjacent: 2 act-table loads per superchunk, not 4
                    hid = hid_t[sc]
                    zbs = []
                    for tt in range(NT):
                        pp = psA.tile([128, HL], F32, tag="lamT",
                                      name=f"ppu{tt}_{sc}")
                        for k2 in range(2):
                            nc.tensor.matmul(
                                pp[:],
                                hid[k2][:, tt * 128:(tt + 1) * 128],
                                cs2[:, k2 * HL:(k2 + 1) * HL],
                                start=(k2 == 0), stop=(k2 == 1))
                        zb = small.tile([128, HL], F32, tag=f"zb{tt}",
                                        name=f"zb{tt}_{sc}")
                        nc.vector.tensor_add(zb[:], pp[:], b2[:])
                        zbs.append(zb)
                    lcs = []
                    for tt in range(NT):
                        lc = small.tile([128, HL], F32, tag=f"lc{tt}",
                                        name=f"lc{tt}_{sc}")
                        nc.scalar.activation(lc[:], zbs[tt][:], AF.Sigmoid)
                        lcs.append(lc)
                    for tt in range(NT):
                        uu = upool.tile([128, HL], F32, tag=f"u{tt}",
                                        name=f"uu{tt}_{sc}")
                        nc.scalar.activation(uu[:], lcs[tt][:], AF.Ln,
                                             bias=1.0, scale=0.2)
                        u_tt_t[sc][tt] = uu

                steps.append(u_step)
                return steps

            def oproj_steps(sc):
                steps = []

                def o_step(tt, no):
                    def go():
                        OT = OT_t[sc]
                        pp = psB.tile([128, 512], F32, tag="proj",
                                      name=f"ppo{tt}{no}_{sc}")
                        for mo in range(NP):
                            nc.tensor.matmul(
                                pp[:],
                                OT[mo][:, tt * 128:(tt + 1) * 128],
                                wo[:, mo * D + no * 512: mo * D + no * 512 + 512],
                                start=(mo == 0), stop=(mo == NP - 1))
                        ysb = ypool.tile([128, 512], F32, tag=f"y{no}",
                                         name=f"y{tt}{no}_{sc}")
                        if no == 0:
                            nc.vector.tensor_copy(ysb[:], pp[:])
                        else:
                            nc.scalar.activation(ysb[:], pp[:], AF.Copy)
                        nc.sync.dma_start(
                            y_d[sc * TC + tt * 128: sc * TC + (tt + 1) * 128,
                                no * 512:(no + 1) * 512],
                            ysb[:])
                    return go

                for tt in range(NT):
                    for no in range(2):
                        steps.append(o_step(tt, no))
                return steps

            def rec_pair_chunk(sc, tt, mo):
                q_p = q_et_t[sc][mo][:, tt * 128:(tt + 1) * 128]
                k_p = k_et_t[sc][mo][:, tt * 128:(tt + 1) * 128]
                vbf_p = v_bf_t[sc][tt][:, mo * 128:(mo + 1) * 128]
                OT = OT_t[sc]
                u_tt = u_tt_t[sc]
                # log-lambda [t, (2x64 e)] fp16; L[e,t] = lam^T @ tri
                lam = rec.tile([128, 128], F16, tag="lam",
                               name=f"lam{sc}_{tt}_{mo}")
                for j in range(2):
                    h = 2 * mo + j
                    nc.vector.tensor_scalar(
                        lam[:, j * 64:(j + 1) * 64],
                        lb[:, h * 64:(h + 1) * 64],
                        u_tt[tt][:, h:h + 1], LOGCLIP,
                        AL.add, AL.min)
                L = psA.tile([128, 128], F32, tag="lamT",
                             name=f"L{sc}_{tt}_{mo}")
                nc.tensor.matmul(L[:], lam[:], triH[:],
                                 start=True, stop=True)

                L127 = L[:, 127:128]
                ccol = small.tile([128, 1], F32, tag="ccol",
                                  name=f"cc{sc}_{tt}_{mo}")
                nc.vector.tensor_scalar_mul(ccol[:], L127, 0.5)
                cneg = small.tile([128, 1], F32, tag="cneg",
                                  name=f"cn{sc}_{tt}_{mo}")
                nc.vector.tensor_scalar_mul(cneg[:], L127, -0.5)
                ec = small.tile([128, 1], F32, tag="ec",
                                name=f"ec{sc}_{tt}_{mo}")
                nc.scalar.activation(ec[:], L127, AF.Exp, scale=0.5)
                eq = rec.tile([128, 128], F32, tag="eq",
                              name=f"eq{sc}_{tt}_{mo}")
                nc.scalar.activation(eq[:], L[:], AF.Exp, bias=cneg[:])
                ekc = rec.tile([128, 128], F32, tag="ekc",
                               name=f"ekc{sc}_{tt}_{mo}")
                nc.scalar.activation(ekc[:], L[:], AF.Exp, bias=ccol[:],
                                     scale=-1.0)

                qt = rec.tile([128, 128], BF16, tag="qt",
                              name=f"qt{sc}_{tt}_{mo}")
                nc.vector.tensor_mul(qt[:], q_p, eq[:])
                kt = rec.tile([128, 128], BF16, tag="kt",
                              name=f"kt{sc}_{tt}_{mo}")
                nc.vector.tensor_mul(kt[:], k_p, ekc[:])
                kh = rec.tile([128, 128], F32, tag="kh",
                              name=f"kh{sc}_{tt}_{mo}")
                nc.vector.tensor_scalar_mul(kh[:], kt[:], ec[:])

                # K-hat pair transpose -> [t, (2x64 e)]
                khT = psA.tile([128, 128], F32, tag="khT",
                               name=f"khT{sc}_{tt}_{mo}")
                nc.tensor.transpose(khT[:], kh[:], ident[:])
                khTs = rec.tile([128, 128], BF16, tag="khTs",
                                name=f"khTs{sc}_{tt}_{mo}")
                nc.scalar.activation(khTs[:], khT[:], AF.Copy)

                # S_scaled (block-diagonal, both heads)
                ssc = rec.tile([128, 128], BF16, tag="ssc",
                               name=f"ssc{sc}_{tt}_{mo}")
                nc.vector.tensor_scalar_mul(ssc[:], S[mo][:], ec[:])

                # state delta for the pair (block-diagonal valid)
                sd = psA.tile([128, 128], F32, tag="sd",
                              name=f"sd{sc}_{tt}_{mo}")
                nc.tensor.matmul(sd[:], khTs[:], vbf_p, start=True,
                                 stop=True)

                op = psB.tile([128, 128], F32, tag="outT",
                              name=f"op{sc}_{tt}_{mo}")
                for j in range(2):
                    sl = slice(j * 64, (j + 1) * 64)
                    at = psA.tile([128, 128], F32, tag="at",
                                  name=f"at{sc}_{tt}_{mo}_{j}")
                    nc.tensor.matmul(at[:], kt[sl, :], qt[sl, :],
                                     start=True, stop=True)
                    atm = rec.tile([128, 128], BF16, tag="atm",
                                   name=f"atm{sc}_{tt}_{mo}_{j}")
                    nc.vector.select(atm[:], tri_u8[:], at[:], z128[:])

                    nc.tensor.matmul(op[sl, :],
                                     vbf_p[:, j * 64:(j + 1) * 64],
                                     atm[:], start=True, stop=False,
                                     skip_group_check=True)
                # inter-chunk term for BOTH heads via block-diag ssc
                nc.tensor.matmul(op[:], ssc[:], qt[:], start=False,
                                 stop=True, skip_group_check=True)
                nc.scalar.activation(
                    OT[mo][:, tt * 128:(tt + 1) * 128], op[:], AF.Copy)
                for j in range(2):
                    sl = slice(j * 64, (j + 1) * 64)
                    cs = slice(j * 64, (j + 1) * 64)
                    # S = ec*(ssc) + sd = ec^2*S + sd (per head block)
                    nc.vector.scalar_tensor_tensor(
                        S[mo][sl, cs], ssc[sl, cs], ec[sl, :],
                        sd[sl, cs], AL.mult, AL.add)

            # ================= main schedule =================
            load_x(0)
            for st in proj_steps(0):
                st()

            for sc in range(SC):
                OT_t[sc] = [otpool.tile([128, TC], F16, tag=f"ot{mo}",
                                        name=f"OT{mo}_{sc}")
                            for mo in range(NP)]
                if sc + 1 < SC:
                    load_x(sc + 1)
                inject = []
                if sc >= 1:
                    inject += oproj_steps(sc - 1)
                if sc + 1 < SC:
                    inject += proj_steps(sc + 1)
                n_slots = NT * NP
                per = [len(inject) * (i + 1) // n_slots for i in range(n_slots)]
                done = 0
                slot = 0
                for tt in range(NT):
                    for mo in range(NP):
                        rec_pair_chunk(sc, tt, mo)
                        while done < per[slot]:
                            inject[done]()
                            done += 1
                        slot += 1

            for st in oproj_steps(SC - 1):
                st()
    _split_multiwait(nc)
    return nc


_NC = None

def _get_nc():
    global _NC
    if _NC is None:
        _NC = _build()
    return _NC


def _sigmoid(x):
    return 1.0 / (1.0 + np.exp(-x))


def kernel(x, q_w, k_w, v_w, o_w, cs_w1, cs_b1, cs_w2, cs_b2, decay_params):
    x = np.asarray(x, np.float32)
    nc = _get_nc()

    def wlay(wT_cols):  # [1024, M] -> [128, 8*M] (dt-major along free)
        return np.ascontiguousarray(
            wT_cols.reshape(NDT, 128, wT_cols.shape[1]).transpose(1, 0, 2)
            .reshape(128, -1))

    qwT = np.asarray(q_w, np.float32).T
    kwT = np.asarray(k_w, np.float32).T
    vwT = np.asarray(v_w, np.float32).T
    owT = np.asarray(o_w, np.float32).T
    cs1T = np.asarray(cs_w1, np.float32).T      # [1024, 256]
    cs2T = np.asarray(cs_w2, np.float32).T      # [256, 16]
    lbase = np.log(_sigmoid(np.asarray(decay_params, np.float32)))  # [H, E]
    b1c = np.ascontiguousarray(np.asarray(cs_b1, np.float32).reshape(2, 128).T)

    in_maps = []
    for i in range(8):
        b, g = i // 2, i % 2
        hs = g * HL
        xT = x[b].T                                            # [1024, 2048]
        xTl = np.ascontiguousarray(
            xT.reshape(NDT, 128, T).transpose(1, 0, 2).reshape(128, NDT * T))
        wo_loc = owT[hs * E:(hs + HL) * E, :]                  # [512, 1024]
        wol = np.ascontiguousarray(                            # [128, 4*1024]
            wo_loc.reshape(4, 128, D).transpose(1, 0, 2).reshape(128, 4 * D))
        cs2l = np.ascontiguousarray(
            cs2T[:, hs:hs + HL].reshape(2, 128, HL).transpose(1, 0, 2)
            .reshape(128, 2 * HL))
        in_maps.append({
            "xT": xTl,
            "wq": wlay(qwT[:, hs * E:(hs + HL) * E]),
            "wk": wlay(kwT[:, hs * E:(hs + HL) * E]),
            "wv": wlay(vwT[:, hs * E:(hs + HL) * E]),
            "wo": wol,
            "cs1": wlay(cs1T),
            "cs2": cs2l,
            "b1": b1c,
            "b2": np.ascontiguousarray(
                np.broadcast_to(np.asarray(cs_b2, np.float32)[hs:hs + HL],
                                (128, HL))),
            "lb": np.ascontiguousarray(
                np.broadcast_to(lbase[hs:hs + HL].reshape(1, DL), (128, DL))),
        })

    res = run_bass_kernel_spmd(nc, in_maps, core_ids=list(range(8)))
    global LAST_RESULT
    LAST_RESULT = res
    y = np.empty((B, T, D), np.float32)
    for b in range(B):
        y[b] = res.results[2 * b]["y"] + res.results[2 * b + 1]["y"]
    return y



# revision 10
# speedup vs baseline: 1.7437x; 1.0803x over previous
"""MobiuAttention Trainium2 kernel (8 NeuronCores, SPMD).

Sharding: core i handles (batch b = i//2, head-group g = i%2) -> 8 local heads.
Per core: fp32r projections, complexity sensor, chunked linear-attention
recurrence (chunk C=128, log-space cumulative decay, head-PAIR packed on the
128 partitions, fp32 matmuls), o_proj partial with the local head-slice of
o_w. Host sums the two partial y's per batch.
"""
import sys
sys.path.insert(0, '/opt/trn_rl_repo')

import numpy as np
import bass_rust
import concourse.bass as bass
import concourse.mybir as mybir
import concourse.tile as tile
from concourse.bass_utils import run_bass_kernel_spmd
from concourse.masks import make_identity, make_upper_triangular

F32 = mybir.dt.float32
F32R = mybir.dt.float32r
U8 = mybir.dt.uint8
BF16 = mybir.dt.bfloat16
AL = mybir.AluOpType
AF = mybir.ActivationFunctionType

B, T, D, H, E = 4, 2048, 1024, 16, 64
DH = D // 4          # 256 sensor hidden
HL = 8               # heads per core
NP = HL // 2         # 4 head pairs
DL = HL * E          # 512 local head dim
SC = 8               # superchunks
TC = T // SC         # 256 tokens per superchunk
C = 128              # recurrence chunk
NT = TC // C         # 2 chunks per superchunk
NDT = D // 128       # 8 contraction tiles
LOGCLIP = float(np.log(0.9995))

SEQ_ENGINES = {mybir.EngineType.PE, mybir.EngineType.DVE, mybir.EngineType.Activation,
               mybir.EngineType.Pool, mybir.EngineType.SP}


def _split_multiwait(nc, max_waits=1):
    """Walrus here encodes at most one sync-wait per instruction; hoist extra
    waits onto single-wait NOPs just before, on the same in-order sequencer."""
    for f in nc.m.functions:
        for bb in f.blocks:
            changed = False
            newlist = []
            for inst in bb.instructions:
                si = inst.sync_info
                if (si is not None and len(si.on_wait) > max_waits
                        and inst.engine in SEQ_ENGINES):
                    waits = list(si.on_wait)
                    for w in waits[:-1]:
                        nop = mybir.InstNoOp(name=nc.get_next_instruction_name(),
                                             ins=[], outs=[])
                        nop.engine = inst.engine
                        nop.sync_info = bass_rust.SyncInfo(on_wait=[w], on_update=[])
                        newlist.append(nop)
                        nc.register_instruction(nop)
                    inst.sync_info = bass_rust.SyncInfo(
                        on_wait=[waits[-1]], on_update=list(si.on_update))
                    changed = True
                newlist.append(inst)
            if changed:
                bb.instructions = newlist


def _build():
    nc = bass.Bass(trn_type="TRN2", num_devices=8)
    xT_d = nc.dram_tensor("xT", [128, NDT * T], F32R, kind="ExternalInput")
    wq_d = nc.dram_tensor("wq", [128, NDT * DL], F32R, kind="ExternalInput")
    wk_d = nc.dram_tensor("wk", [128, NDT * DL], F32R, kind="ExternalInput")
    wv_d = nc.dram_tensor("wv", [128, NDT * DL], F32R, kind="ExternalInput")
    wo_d = nc.dram_tensor("wo", [128, 4 * D], F32R, kind="ExternalInput")
    cs1_d = nc.dram_tensor("cs1", [128, NDT * DH], F32R, kind="ExternalInput")
    cs2_d = nc.dram_tensor("cs2", [128, 2 * HL], F32R, kind="ExternalInput")
    b1_d = nc.dram_tensor("b1", [128, 2], F32, kind="ExternalInput")
    b2_d = nc.dram_tensor("b2", [128, HL], F32, kind="ExternalInput")
    lb_d = nc.dram_tensor("lb", [128, DL], F32, kind="ExternalInput")
    y_d = nc.dram_tensor("y", [T, D], F32, kind="ExternalOutput")

    with tile.TileContext(nc) as tc:
        with tc.tile_pool(name="wpool", bufs=1) as wpool, \
             tc.tile_pool(name="cpool", bufs=1) as cpool, \
             tc.tile_pool(name="state", bufs=1) as state, \
             tc.tile_pool(name="xpool", bufs=2) as xpool, \
             tc.tile_pool(name="qkv", bufs=2) as qkv, \
             tc.tile_pool(name="hpool", bufs=2) as hpool, \
             tc.tile_pool(name="upool", bufs=2) as upool, \
             tc.tile_pool(name="otpool", bufs=2) as otpool, \
             tc.tile_pool(name="ypool", bufs=2) as ypool, \
             tc.tile_pool(name="rec", bufs=3) as rec, \
             tc.tile_pool(name="small", bufs=4) as small, \
             tc.tile_pool(name="psA", bufs=1, space="PSUM") as psA, \
             tc.tile_pool(name="psB", bufs=2, space="PSUM") as psB:

            # ---- constants ----
            ident = cpool.tile([128, 128], F32)
            make_identity(nc, ident[:])
            tri = cpool.tile([128, 128], F32)
            make_upper_triangular(nc, tri[:], val=1.0, diag=True)
            tri_u8 = cpool.tile([128, 128], U8)
            nc.vector.tensor_copy(tri_u8[:], tri[:])
            triH = cpool.tile([128, 128], F16)
            nc.vector.tensor_copy(triH[:], tri[:])
            z128 = cpool.tile([128, 128], F32)
            nc.vector.memset(z128[:], 0.0)

            # ---- weights ----
            wq = wpool.tile([128, NDT * DL], F32R)
            nc.sync.dma_start(wq[:], wq_d[:])
            wk = wpool.tile([128, NDT * DL], F32R)
            nc.sync.dma_start(wk[:], wk_d[:])
            wv = wpool.tile([128, NDT * DL], F32R)
            nc.sync.dma_start(wv[:], wv_d[:])
            wo = wpool.tile([128, 4 * D], F32R)
            nc.sync.dma_start(wo[:], wo_d[:])
            cs1 = wpool.tile([128, NDT * DH], F32R)
            nc.sync.dma_start(cs1[:], cs1_d[:])
            cs2 = wpool.tile([128, 2 * HL], F32R)
            nc.sync.dma_start(cs2[:], cs2_d[:])
            b1 = wpool.tile([128, 2], F32)
            nc.sync.dma_start(b1[:], b1_d[:])
            b2 = wpool.tile([128, HL], F32)
            nc.sync.dma_start(b2[:], b2_d[:])
            lb = wpool.tile([128, DL], F32)
            nc.sync.dma_start(lb[:], lb_d[:])

            # ---- per-pair recurrent state [ (h0 e | h1 e), f ] ----
            S = []
            for mo in range(NP):
                sh = state.tile([128, 128], F32, tag=f"S{mo}", name=f"S{mo}")
                nc.vector.memset(sh[:], 0.0)
                S.append(sh)

            # ---- per-superchunk tile stores ----
            xt_t, q_et_t, k_et_t, v_bf_t = {}, {}, {}, {}
            hid_t, u_tt_t, OT_t = {}, {}, {}

            def load_x(sc):
                xt = xpool.tile([128, NDT * TC], F16, tag="xt", name=f"xt{sc}")
                for dt in range(NDT):
                    nc.sync.dma_start(
                        xt[:, dt * TC:(dt + 1) * TC],
                        xT_d[:, dt * T + sc * TC: dt * T + sc * TC + TC])
                xt_t[sc] = xt

            def proj_steps(sc):
                steps = []
                q_et_t[sc] = [None] * NP
                k_et_t[sc] = [None] * NP
                v_bf_t[sc] = [None] * NT
                hid_t[sc] = [None] * 2
                u_tt_t[sc] = [None] * NT

                def qk_step(name, w, store, mo):
                    def go():
                        xt = xt_t[sc]
                        pp = psB.tile([128, TC], F32, tag="proj",
                                      name=f"pp{name}{mo}_{sc}")
                        for dt in range(NDT):
                            nc.tensor.matmul(
                                pp[:],
                                w[:, dt * DL + mo * 128: dt * DL + (mo + 1) * 128],
                                xt[:, dt * TC:(dt + 1) * TC],
                                start=(dt == 0), stop=(dt == NDT - 1))
                        sb = qkv.tile([128, TC], F16, tag=f"{name}{mo}",
                                      name=f"{name}{mo}_{sc}")
                        nc.vector.tensor_copy(sb[:], pp[:])
                        store[sc][mo] = sb
                    return go

                qs = [qk_step("q", wq, q_et_t, mo) for mo in range(NP)]
                ks = [qk_step("k", wk, k_et_t, mo) for mo in range(NP)]

                def v_step(tt):
                    def go():
                        xt = xt_t[sc]
                        pp = psB.tile([128, DL], F32, tag="proj",
                                      name=f"ppv{tt}_{sc}")
                        for dt in range(NDT):
                            nc.tensor.matmul(
                                pp[:, 0:DL],
                                xt[:, dt * TC + tt * 128: dt * TC + (tt + 1) * 128],
                                wv[:, dt * DL:(dt + 1) * DL],
                                start=(dt == 0), stop=(dt == NDT - 1))
                        vb = qkv.tile([128, DL], BF16, tag=f"vb{tt}",
                                      name=f"vb{tt}_{sc}")
                        nc.vector.tensor_copy(vb[:], pp[:, 0:DL])
                        v_bf_t[sc][tt] = vb
                    return go

                vs = [v_step(tt) for tt in range(NT)]

                def hid_step(mo):
                    def go():
                        xt = xt_t[sc]
                        pp = psB.tile([128, TC], F32, tag="proj",
                                      name=f"pph{mo}_{sc}")
                        for dt in range(NDT):
                            nc.tensor.matmul(
                                pp[:],
                                cs1[:, dt * DH + mo * 128: dt * DH + (mo + 1) * 128],
                                xt[:, dt * TC:(dt + 1) * TC],
                                start=(dt == 0), stop=(dt == NDT - 1))
                        sb = hpool.tile([128, TC], F16, tag=f"h{mo}",
                                        name=f"h{mo}_{sc}")
                        nc.scalar.activation(sb[:], pp[:], AF.Tanh,
                                             bias=b1[:, mo:mo + 1])
                        hid_t[sc][mo] = sb
                    return go



                def u_step():
                    # u-mms for both chunks, then Sigmoids adjacent, then Lns
                    # adjacent: 2 act-table loads per superchunk, not 4
                    hid = hid_t[sc]
                    zbs = []
                    for tt in range(NT):
                        pp = psA.tile([128, HL], F32, tag="lamT",
                                      name=f"ppu{tt}_{sc}")
                        for k2 in range(2):
                            nc.tensor.matmul(
                                pp[:],
                                hid[k2][:, tt * 128:(tt + 1) * 128],
                                cs2[:, k2 * HL:(k2 + 1) * HL],
                                start=(k2 == 0), stop=(k2 == 1))
                        zb = small.tile([128, HL], F32, tag=f"zb{tt}",
                                        name=f"zb{tt}_{sc}")
                        nc.vector.tensor_add(zb[:], pp[:], b2[:])
                        zbs.append(zb)
                    lcs = []
                    for tt in range(NT):
                        lc = small.tile([128, HL], F32, tag=f"lc{tt}",
                                        name=f"lc{tt}_{sc}")
                        nc.scalar.activation(lc[:], zbs[tt][:], AF.Sigmoid)
                        lcs.append(lc)
                    for tt in range(NT):
                        uu = upool.tile([128, HL], F32, tag=f"u{tt}",
                                        name=f"uu{tt}_{sc}")
                        nc.scalar.activation(uu[:], lcs[tt][:], AF.Ln,
                                             bias=1.0, scale=0.2)
                        u_tt_t[sc][tt] = uu

                h0, h1 = hid_step(0), hid_step(1)

                def sensor_combo():
                    # Tanh,Tanh then Sigmoid,Sigmoid then Ln,Ln adjacent on
                    # the scalar queue: 2 act-table loads per superchunk
                    h0(); h1(); u_step()

                # critical inputs of the next superchunk's first pair-chunks
                # come first; remaining pairs follow
                steps = [qs[0], ks[0], vs[0], sensor_combo,
                         qs[1], ks[1], vs[1],
                         qs[2], ks[2], qs[3], ks[3]]
                return steps

            def oproj_steps(sc):
                steps = []

                def o_step(tt, no):
                    def go():
                        OT = OT_t[sc]
                        pp = psB.tile([128, 512], F32, tag="proj",
                                      name=f"ppo{tt}{no}_{sc}")
                        for mo in range(NP):
                            nc.tensor.matmul(
                                pp[:],
                                OT[mo][:, tt * 128:(tt + 1) * 128],
                                wo[:, mo * D + no * 512: mo * D + no * 512 + 512],
                                start=(mo == 0), stop=(mo == NP - 1))
                        ysb = ypool.tile([128, 512], F32, tag=f"y{no}",
                                         name=f"y{tt}{no}_{sc}")
                        if no == 0:
                            nc.vector.tensor_copy(ysb[:], pp[:])
                        else:
                            nc.scalar.activation(ysb[:], pp[:], AF.Copy)
                        nc.sync.dma_start(
                            y_d[sc * TC + tt * 128: sc * TC + (tt + 1) * 128,
                                no * 512:(no + 1) * 512],
                            ysb[:])
                    return go

                for tt in range(NT):
                    for no in range(2):
                        steps.append(o_step(tt, no))
                return steps

            def rec_pair_chunk(sc, tt, mo):
                q_p = q_et_t[sc][mo][:, tt * 128:(tt + 1) * 128]
                k_p = k_et_t[sc][mo][:, tt * 128:(tt + 1) * 128]
                vbf_p = v_bf_t[sc][tt][:, mo * 128:(mo + 1) * 128]
                OT = OT_t[sc]
                u_tt = u_tt_t[sc]
                # log-lambda [t, (2x64 e)] fp16; L[e,t] = lam^T @ tri
                lam = rec.tile([128, 128], F16, tag="lam",
                               name=f"lam{sc}_{tt}_{mo}")
                for j in range(2):
                    h = 2 * mo + j
                    nc.vector.tensor_scalar(
                        lam[:, j * 64:(j + 1) * 64],
                        lb[:, h * 64:(h + 1) * 64],
                        u_tt[tt][:, h:h + 1], LOGCLIP,
                        AL.add, AL.min)
                L = psA.tile([128, 128], F32, tag="lamT",
                             name=f"L{sc}_{tt}_{mo}")
                nc.tensor.matmul(L[:], lam[:], triH[:],
                                 start=True, stop=True)

                L127 = L[:, 127:128]
                ccol = small.tile([128, 1], F32, tag="ccol",
                                  name=f"cc{sc}_{tt}_{mo}")
                nc.vector.tensor_scalar_mul(ccol[:], L127, 0.5)
                cneg = small.tile([128, 1], F32, tag="cneg",
                                  name=f"cn{sc}_{tt}_{mo}")
                nc.vector.tensor_scalar_mul(cneg[:], L127, -0.5)
                ec = small.tile([128, 1], F32, tag="ec",
                                name=f"ec{sc}_{tt}_{mo}")
                nc.scalar.activation(ec[:], L127, AF.Exp, scale=0.5)
                eq = rec.tile([128, 128], BF16, tag="eq",
                              name=f"eq{sc}_{tt}_{mo}")
                nc.scalar.activation(eq[:], L[:], AF.Exp, bias=cneg[:])
                ekc = rec.tile([128, 128], BF16, tag="ekc",
                               name=f"ekc{sc}_{tt}_{mo}")
                nc.scalar.activation(ekc[:], L[:], AF.Exp, bias=ccol[:],
                                     scale=-1.0)

                qt = rec.tile([128, 128], BF16, tag="qt",
                              name=f"qt{sc}_{tt}_{mo}")
                nc.vector.tensor_mul(qt[:], q_p, eq[:])
                kt = rec.tile([128, 128], BF16, tag="kt",
                              name=f"kt{sc}_{tt}_{mo}")
                nc.vector.tensor_mul(kt[:], k_p, ekc[:])
                kh = rec.tile([128, 128], F32, tag="kh",
                              name=f"kh{sc}_{tt}_{mo}")
                nc.vector.tensor_scalar_mul(kh[:], kt[:], ec[:])

                # K-hat pair transpose -> [t, (2x64 e)]
                khT = psA.tile([128, 128], F32, tag="khT",
                               name=f"khT{sc}_{tt}_{mo}")
                nc.tensor.transpose(khT[:], kh[:], ident[:])
                khTs = rec.tile([128, 128], BF16, tag="khTs",
                                name=f"khTs{sc}_{tt}_{mo}")
                nc.scalar.activation(khTs[:], khT[:], AF.Copy)

                # S_scaled (block-diagonal, both heads)
                ssc = rec.tile([128, 128], BF16, tag="ssc",
                               name=f"ssc{sc}_{tt}_{mo}")
                nc.vector.tensor_scalar_mul(ssc[:], S[mo][:], ec[:])

                # state delta for the pair (block-diagonal valid)
                sd = psA.tile([128, 128], F32, tag="sd",
                              name=f"sd{sc}_{tt}_{mo}")
                nc.tensor.matmul(sd[:], khTs[:], vbf_p, start=True,
                                 stop=True)

                op = psB.tile([128, 128], F32, tag="outT",
                              name=f"op{sc}_{tt}_{mo}")
                for j in range(2):
                    sl = slice(j * 64, (j + 1) * 64)
                    at = psA.tile([128, 128], F32, tag="at",
                                  name=f"at{sc}_{tt}_{mo}_{j}")
                    nc.tensor.matmul(at[:], kt[sl, :], qt[sl, :],
                                     start=True, stop=True)
                    atm = rec.tile([128, 128], BF16, tag="atm",
                                   name=f"atm{sc}_{tt}_{mo}_{j}")
                    nc.vector.select(atm[:], tri_u8[:], at[:], z128[:])

                    nc.tensor.matmul(op[sl, :],
                                     vbf_p[:, j * 64:(j + 1) * 64],
                                     atm[:], start=True, stop=False,
                                     skip_group_check=True)
                # inter-chunk term for BOTH heads via block-diag ssc
                nc.tensor.matmul(op[:], ssc[:], qt[:], start=False,
                                 stop=True, skip_group_check=True)
                nc.scalar.activation(
                    OT[mo][:, tt * 128:(tt + 1) * 128], op[:], AF.Copy)
                for j in range(2):
                    sl = slice(j * 64, (j + 1) * 64)
                    cs = slice(j * 64, (j + 1) * 64)
                    # S = ec*(ssc) + sd = ec^2*S + sd (per head block)
                    nc.vector.scalar_tensor_tensor(
                        S[mo][sl, cs], ssc[sl, cs], ec[sl, :],
                        sd[sl, cs], AL.mult, AL.add)

            # ================= main schedule =================
            load_x(0)
            for st in proj_steps(0):
                st()

            for sc in range(SC):
                OT_t[sc] = [otpool.tile([128, TC], F16, tag=f"ot{mo}",
                                        name=f"OT{mo}_{sc}")
                            for mo in range(NP)]
                if sc + 1 < SC:
                    load_x(sc + 1)
                inject = []
                if sc + 1 < SC:
                    inject += proj_steps(sc + 1)
                if sc >= 1:
                    inject += oproj_steps(sc - 1)
                n_slots = NT * NP
                per = [len(inject) * (i + 1) // n_slots for i in range(n_slots)]
                done = 0
                slot = 0
                for tt in range(NT):
                    for mo in range(NP):
                        rec_pair_chunk(sc, tt, mo)
                        while done < per[slot]:
                            inject[done]()
                            done += 1
                        slot += 1

            for st in oproj_steps(SC - 1):
                st()
    _split_multiwait(nc)
    return nc


_NC = None

def _get_nc():
    global _NC
    if _NC is None:
        _NC = _build()
    return _NC


def _sigmoid(x):
    return 1.0 / (1.0 + np.exp(-x))


def kernel(x, q_w, k_w, v_w, o_w, cs_w1, cs_b1, cs_w2, cs_b2, decay_params):
    x = np.asarray(x, np.float32)
    nc = _get_nc()

    def wlay(wT_cols):  # [1024, M] -> [128, 8*M] (dt-major along free)
        return np.ascontiguousarray(
            wT_cols.reshape(NDT, 128, wT_cols.shape[1]).transpose(1, 0, 2)
            .reshape(128, -1))

    qwT = np.asarray(q_w, np.float32).T
    kwT = np.asarray(k_w, np.float32).T
    vwT = np.asarray(v_w, np.float32).T
    owT = np.asarray(o_w, np.float32).T
    cs1T = np.asarray(cs_w1, np.float32).T      # [1024, 256]
    cs2T = np.asarray(cs_w2, np.float32).T      # [256, 16]
    lbase = np.log(_sigmoid(np.asarray(decay_params, np.float32)))  # [H, E]
    b1c = np.ascontiguousarray(np.asarray(cs_b1, np.float32).reshape(2, 128).T)

    in_maps = []
    for i in range(8):
        b, g = i // 2, i % 2
        hs = g * HL
        xT = x[b].T                                            # [1024, 2048]
        xTl = np.ascontiguousarray(
            xT.reshape(NDT, 128, T).transpose(1, 0, 2).reshape(128, NDT * T))
        wo_loc = owT[hs * E:(hs + HL) * E, :]                  # [512, 1024]
        wol = np.ascontiguousarray(                            # [128, 4*1024]
            wo_loc.reshape(4, 128, D).transpose(1, 0, 2).reshape(128, 4 * D))
        cs2l = np.ascontiguousarray(
            cs2T[:, hs:hs + HL].reshape(2, 128, HL).transpose(1, 0, 2)
            .reshape(128, 2 * HL))
        in_maps.append({
            "xT": xTl,
            "wq": wlay(qwT[:, hs * E:(hs + HL) * E]),
            "wk": wlay(kwT[:, hs * E:(hs + HL) * E]),
            "wv": wlay(vwT[:, hs * E:(hs + HL) * E]),
            "wo": wol,
            "cs1": wlay(cs1T),
            "cs2": cs2l,
            "b1": b1c,
            "b2": np.ascontiguousarray(
                np.broadcast_to(np.asarray(cs_b2, np.float32)[hs:hs + HL],
                                (128, HL))),
            "lb": np.ascontiguousarray(
                np.broadcast_to(lbase[hs:hs + HL].reshape(1, DL), (128, DL))),
        })

    res = run_bass_kernel_spmd(nc, in_maps, core_ids=list(range(8)))
    global LAST_RESULT
    LAST_RESULT = res
    y = np.empty((B, T, D), np.float32)
    for b in range(B):
        y[b] = res.results[2 * b]["y"] + res.results[2 * b + 1]["y"]
    return y

